# revision 1
# baseline (speedup 1.0000x reference)
"""Trainium2 Bass kernel for nn_CompatibleLearningLoss (MoCo-style queue contrastive loss).

Strategy: shard the queue dimension (Q=32768) across 8 NeuronCores (4096 rows
each).  Each core computes its slice of the three matmuls
    old_embeds  @ feat_queue_shard.T   -> weight
    new_e       @ feat_queue_shard.T   -> scores1
    new_logits  @ logit_queue_shard.T  -> scores2
and reduces per-row partial softmax statistics (chunk max, exp-sum vs chunk
max, masked-weighted raw sums) into a [128, 128] stats tile.  The host combines
the partials in float64 and produces the two scalar losses.

Marshaling (all host-side): circular queue scatter, new_embeds normalization,
label mask, and - crucially - pre-transposition of both queue matrices so the
contraction dim lands on SBUF partitions.  The device then runs pure streaming
matmuls with zero on-chip transposes.  The logit queue is cast to bf16 (halves
the dominant DMA stream; measured loss error ~1e-4 relative); the feat path
uses fp32r multiplies (near-fp32, ~5e-6).
"""

from contextlib import nullcontext

import numpy as np

import concourse.bass as bass
import concourse.tile as tile
from concourse import mybir
from concourse.bass_utils import run_bass_kernel_spmd
from concourse.vector_clock import ScopedClock

N = 128      # batch
D = 512      # embed dim
C = 8192     # logit dim
Q = 32768    # queue length
N_CORES = 8
QS = Q // N_CORES          # 4096 queue rows per core
EPS = 1e-12

F32 = mybir.dt.float32
BF16 = mybir.dt.bfloat16
F32R = mybir.dt.float32r

# stats tile column layout (per core, [128, 128] f32)
# cols 0:8      m1 parts   (feat-path chunk maxes, 8 chunks of 512)
# cols 8:16     z1 parts   (feat-path sum exp(s - chunk max))
# cols 16:24    a1 parts   (feat-path sum maskw * s_raw)
# cols 24:32    W  parts   (sum maskw)
# cols 32:32+B  m2 parts   (logit-path chunk maxes, B_CHUNKS chunks of 512)
# cols 64:64+B  z2 parts
# cols 96:96+B  a2 parts
A_CHUNKS = 8
B_CHUNKS = 8


def _split_excess_waits(nc: bass.Bass, limit: int = 1) -> None:
    """This walrus build rejects instructions carrying more than one sync wait
    ("Too many sync wait commands").  Tile's sem-assignment freely attaches
    several.  Move excess waits onto same-engine nops inserted right before
    the offending instruction (queue order makes that equivalent)."""
    for f in nc.m.functions:
        for bb in f.blocks:
            insts = bb.instructions
            insertions = []
            for idx, inst in enumerate(insts):
                si = inst.sync_info
                if si is None:
                    continue
                cap = 2 if isinstance(inst, mybir.InstEventSemaphore) else limit
                waits = list(si.on_wait)
                if len(waits) <= cap:
                    continue
                keep = waits[:cap]
                excess = waits[cap:]
                si.on_wait = keep
                nops = []
                for w in excess:
                    nop = mybir.InstNoOp(
                        name=nc.get_next_instruction_name(), ins=[], outs=[]
                    )
                    nop.engine = inst.engine
                    nop.sync_info = mybir.SyncInfo(on_wait=[w], on_update=[])
                    nc.register_instruction(nop, overwrite=True)
                    nops.append(nop)
                insertions.append((idx, nops))
            for idx, nops in reversed(insertions):
                for nop in reversed(nops):
                    bb.instructions.insert(idx, nop)


class PatchedTileContext(tile.TileContext):
    """Work around the 1-sync-wait-per-instruction cap in this walrus build:
    the stock TileContext tail drain carries one wait per outstanding proc,
    which codegen rejects ("Too many sync wait commands").  Split the waits
    across single-wait SP nops instead."""

    def _drain_and_barrier(self, tick_clock, wait_clock):
        drain_inst = self.nc.sync.drain()
        wait_clock.add_sem_waits(
            drain_inst.ins, ScopedClock({None: tick_clock.global_clock})
        )
        si = drain_inst.ins.sync_info
        if si is not None and len(si.on_wait) > 1:
            waits = list(si.on_wait)
            si.on_wait = [waits[0]]
            for w in waits[1:]:
                nop = self.nc.sync.nop(nofuse=True, hint="drain_wait_split")
                nop.ins.sync_info = mybir.SyncInfo(on_wait=[w], on_update=[])
        self.nc.all_engine_barrier()
        assert self.sems is not None
        popped = self.nc._tile_sem_poison_stack.pop()
        assert popped is self._sem_poison
        self.nc.clear_and_free_semaphores(list(self.sems.allocated().values()))
        self.nc.all_engine_barrier()


def _build_program(repeat: int = 1) -> bass.Bass:
    nc = bass.Bass()

    featT = nc.dram_tensor("featT", [D, QS], F32R, kind="ExternalInput")
    logitT = nc.dram_tensor("logitT", [C, QS], BF16, kind="ExternalInput")
    maskh = nc.dram_tensor("maskh", [N, QS], BF16, kind="ExternalInput")
    # stationary operands pre-arranged host-side as their SBUF image
    # [partition, chunk, batch] so each DMA is one contiguous run per partition
    neT = nc.dram_tensor("neT", [128, D // 128, N], F32R, kind="ExternalInput")
    oeT = nc.dram_tensor("oeT", [128, D // 128, N], F32R, kind="ExternalInput")
    nlT = nc.dram_tensor("nlT", [128, C // 128, N], BF16, kind="ExternalInput")
    stats = nc.dram_tensor("stats", [N, 128], F32, kind="ExternalOutput")

    AX = mybir.AxisListType
    OP = mybir.AluOpType
    ACT = mybir.ActivationFunctionType
    CB = C // 128  # 64 contraction chunks (logit path)

    with PatchedTileContext(nc) as tc:
        with (
            tc.tile_pool(name="const", bufs=1) as const,
            tc.tile_pool(name="small", bufs=4) as small,
            tc.tile_pool(name="scr", bufs=2) as scrp,
            tc.tile_pool(name="ftp", bufs=8) as ftp,
        ):
            # replicated stationary operands, pre-transposed host-side
            neT_sb = const.tile([128, D // 128, N], F32R)
            nc.sync.dma_start(out=neT_sb, in_=neT[:, :, :])
            oeT_sb = const.tile([128, D // 128, N], F32R)
            nc.sync.dma_start(out=oeT_sb, in_=oeT[:, :, :])
            nlT_sb = const.tile([128, CB, N], BF16)
            nc.sync.dma_start(out=nlT_sb, in_=nlT[:, :, :])

            mw_sb = const.tile([N, QS], F32)
            mh_sb = const.tile([N, QS], BF16)
            out_sb = const.tile([N, 128], F32)
            s1c_sb = const.tile([N, A_CHUNKS, 512], F32)  # scores1 parked in SBUF

            def stats_block(src, col_m, col_z, col_a, nch, mw3):
                """Per-row stats over a [128, nch, 512] block `src`:
                chunk maxes -> cols [col_m, col_m+nch), exp-sums vs chunk max
                -> cols [col_z, ...), one total masked-weighted raw sum ->
                col_a.  mw3 is the matching [128, nch, 512] maskw view."""
                nc.vector.tensor_reduce(
                    out=out_sb[:, col_m : col_m + nch], in_=src,
                    axis=AX.X, op=OP.max,
                )
                negm = small.tile([128, 8], F32, tag="negm")
                nc.vector.tensor_scalar_mul(
                    out=negm[:, :nch], in0=out_sb[:, col_m : col_m + nch],
                    scalar1=-1.0,
                )
                for k in range(nch):
                    escr = scrp.tile([128, 512], F32, tag="escr")
                    nc.scalar.activation(
                        out=escr, in_=src[:, k, :], func=ACT.Exp,
                        bias=negm[:, k : k + 1], scale=1.0,
                        accum_out=out_sb[:, col_z + k : col_z + k + 1],
                    )
                # a partial = sum maskw * s_raw (product scratch: s1c_sb)
                nc.vector.scalar_tensor_tensor(
                    out=s1c_sb[:, :nch, :], in0=src, scalar=1.0, in1=mw3,
                    op0=OP.mult, op1=OP.mult,
                    accum_out=out_sb[:, col_a : col_a + 1],
                )

            loop_cm = tc.For_i(0, repeat, 1) if repeat > 1 else nullcontext()
            with loop_cm:
                # 0.5*mask (bf16, exact); maskw lands in f32 mw_sb below
                nc.sync.dma_start(out=mh_sb, in_=maskh[:, :])

                # ---- Phase A: feat path (two q halves of 2048, fp32r).
                # PSUM results are parked in SBUF immediately (scores1) or
                # consumed by one DVE op (weight -> maskw) so the banks free
                # fast; the ACT-side stats run later, overlapped with phase B's
                # DMA stream, keeping the two HWDGE rings unobstructed.
                with tc.tile_pool(name="psum_a", bufs=1, space="PSUM") as psum_a:
                    for qh in range(2):
                        ps1 = psum_a.tile([128, 4, 512], F32, tag="ps1")
                        psw = psum_a.tile([128, 4, 512], F32, tag="psw")
                        for dc in range(4):
                            ft = ftp.tile([128, 2048], F32R, tag="ft")
                            dma_eng = nc.sync if dc % 2 == 0 else nc.scalar
                            dma_eng.dma_start(
                                out=ft,
                                in_=featT[dc * 128 : (dc + 1) * 128,
                                          qh * 2048 : (qh + 1) * 2048],
                            )
                            for qw in range(4):
                                rhs = ft[:, qw * 512 : (qw + 1) * 512]
                                nc.tensor.matmul(
                                    ps1[:, qw, :], neT_sb[:, dc, :], rhs,
                                    start=(dc == 0), stop=(dc == 3),
                                )
                                nc.tensor.matmul(
                                    psw[:, qw, :], oeT_sb[:, dc, :], rhs,
                                    start=(dc == 0), stop=(dc == 3),
                                )
                        hs = slice(qh * 2048, (qh + 1) * 2048)
                        # maskw = (w + 1) * 0.5mask  (bf16 mask in, f32 out)
                        nc.vector.scalar_tensor_tensor(
                            out=mw_sb[:, hs].rearrange("p (c q) -> p c q", c=4),
                            in0=psw, scalar=1.0,
                            in1=mh_sb[:, hs].rearrange("p (c q) -> p c q", c=4),
                            op0=OP.add, op1=OP.mult,
                        )
                        # park scores1 in SBUF so the banks free fast
                        nc.vector.tensor_copy(
                            out=s1c_sb[:, qh * 4 : (qh + 1) * 4, :], in_=ps1
                        )

                # ---- Phase B: logit path (bf16, 8 psum accumulators) --------
                with (
                    tc.tile_pool(name="tcp", bufs=8) as tcp,
                    tc.tile_pool(name="psum_b", bufs=1, space="PSUM") as psum_b,
                ):
                    psB = psum_b.tile([128, 8, 512], F32, tag="psB")
                    mw3 = mw_sb.rearrange("p (c q) -> p c q", c=8)

                    def b_step(cb):
                        tcb = tcp.tile([128, QS], BF16, tag="tcb", name="tcb")
                        dma_eng = nc.sync if cb % 2 == 0 else nc.scalar
                        dma_eng.dma_start(
                            out=tcb, in_=logitT[cb * 128 : (cb + 1) * 128, :]
                        )
                        for qw in range(8):
                            nc.tensor.matmul(
                                psB[:, qw, :], nlT_sb[:, cb, :],
                                tcb[:, qw * 512 : (qw + 1) * 512],
                                start=(cb == 0), stop=(cb == CB - 1),
                            )

                    for cb in range(24):
                        b_step(cb)
                    # deferred phase A stats, interleaved mid-stream so the
                    # DVE/ACT work hides under the logit DMA stream
                    nc.vector.tensor_reduce(
                        out=out_sb[:, 24:32], in_=mw3, axis=AX.X, op=OP.add,
                    )
                    stats_block(s1c_sb, 0, 8, 16, A_CHUNKS, mw3)
                    for cb in range(24, CB):
                        b_step(cb)
                    # phase B stats
                    stats_block(psB, 32, 64, 96, 8, mw3)

            nc.sync.dma_start(out=stats[:, :], in_=out_sb)

    _split_excess_waits(nc)
    return nc


_PROGRAM: bass.Bass | None = None
LAST_RESULTS = None  # BassKernelResults of the most recent run (for profiling)


def _get_program() -> bass.Bass:
    global _PROGRAM
    if _PROGRAM is None:
        _PROGRAM = _build_program()
    return _PROGRAM


def _transpose_cast_bf16(a: np.ndarray) -> np.ndarray:
    """[R, C] f32 -> [C, R] bf16 (ml_dtypes), contiguous.  Torch does the
    blocked transpose ~5x faster than numpy; fall back to numpy if absent."""
    import ml_dtypes

    try:
        import torch

        t = torch.from_numpy(np.ascontiguousarray(a))
        lt = t.to(torch.bfloat16).t().contiguous()
        return lt.view(torch.int16).numpy().view(ml_dtypes.bfloat16)
    except ImportError:
        return np.ascontiguousarray(a.astype(ml_dtypes.bfloat16).T)


def host_prep(old_embeds, old_logits, new_embeds, new_logits, labels,
              feat_queue, logit_queue, queue_labels, header):
    """Scatter + normalize + mask + pre-transpose on host; returns per-core
    in_maps and the per-row positive counts M."""
    import ml_dtypes

    old_embeds = np.asarray(old_embeds, dtype=np.float32)
    old_logits = np.asarray(old_logits, dtype=np.float32)
    new_embeds = np.asarray(new_embeds, dtype=np.float32)
    new_logits = np.asarray(new_logits, dtype=np.float32)
    feat_queue = np.array(feat_queue, dtype=np.float32)   # copies (scattered below)
    logit_queue = np.array(logit_queue, dtype=np.float32)
    labels_np = np.asarray(labels).astype(np.int64)
    queue_labels_np = np.asarray(queue_labels).astype(np.int64)
    hdr = int(np.asarray(header))

    n = old_embeds.shape[0]
    q = feat_queue.shape[0]
    assert (n, q) == (N, Q)

    # circular queue scatter
    idx = (hdr + np.arange(n)) % q
    feat_queue[idx] = old_embeds
    logit_queue[idx] = old_logits
    queue_labels_np[idx] = labels_np

    # normalize new_embeds (f64 intermediate, f32 result)
    ne64 = new_embeds.astype(np.float64)
    norm = np.sqrt((ne64 * ne64).sum(axis=1, keepdims=True))
    new_e = (ne64 / np.maximum(norm, EPS)).astype(np.float32)

    # label mask (host): maskh = 0.5 * mask
    mask = (queue_labels_np[None, :] == labels_np[:, None])
    M = mask.sum(axis=1).astype(np.float64)               # [N], >= 1 by construction
    maskh = (0.5 * mask.astype(np.float32)).astype(ml_dtypes.bfloat16)

    def _sbuf_image(aT):
        # [K, N] -> [128, K//128, N] partition-major SBUF image
        k = aT.shape[0]
        return np.ascontiguousarray(
            aT.reshape(k // 128, 128, aT.shape[1]).transpose(1, 0, 2)
        )

    neT = _sbuf_image(np.ascontiguousarray(new_e.T))          # [128, 4, N] f32
    oeT = _sbuf_image(np.ascontiguousarray(old_embeds.T))     # [128, 4, N] f32
    nlT_b = _sbuf_image(_transpose_cast_bf16(new_logits))     # [128, 64, N] bf16

    in_maps = []
    for d in range(N_CORES):
        sl = slice(d * QS, (d + 1) * QS)
        in_maps.append({
            "featT": np.ascontiguousarray(feat_queue[sl].T),   # [D, QS] f32
            "logitT": _transpose_cast_bf16(logit_queue[sl]),   # [C, QS] bf16
            "maskh": np.ascontiguousarray(maskh[:, sl]),
            "neT": neT,
            "oeT": oeT,
            "nlT": nlT_b,
        })
    return in_maps, M


def combine_stats(parts: np.ndarray, M: np.ndarray):
    """parts: [n_cores, 128, 128] f32 stats tiles -> (l1, l2) f32 scalars."""
    parts = parts.astype(np.float64)
    m1p = parts[:, :, 0:8]
    z1p = parts[:, :, 8:16]
    a1p = parts[:, :, 16:17]
    wp = parts[:, :, 24:32]
    m2p = parts[:, :, 32 : 32 + B_CHUNKS]
    z2p = parts[:, :, 64 : 64 + B_CHUNKS]
    a2p = parts[:, :, 96:97]

    W = wp.sum(axis=(0, 2))                               # [N]
    A1 = a1p.sum(axis=(0, 2))
    A2 = a2p.sum(axis=(0, 2))
    m1 = m1p.max(axis=(0, 2))
    m2 = m2p.max(axis=(0, 2))
    Z1 = (z1p * np.exp(m1p - m1[None, :, None])).sum(axis=(0, 2))
    Z2 = (z2p * np.exp(m2p - m2[None, :, None])).sum(axis=(0, 2))

    # sum_j maskw * log_prob = A_raw - (m + log Z) * W ; divide by count, mean, negate
    l1 = -np.mean((A1 - (m1 + np.log(Z1)) * W) / M)
    l2 = -np.mean((A2 - (m2 + np.log(Z2)) * W) / M)
    return (np.float32(l1), np.float32(l2))


def kernel(old_embeds, old_logits, new_embeds, new_logits, labels,
           feat_queue, logit_queue, queue_labels, header):
    global LAST_RESULTS
    in_maps, M = host_prep(
        old_embeds, old_logits, new_embeds, new_logits, labels,
        feat_queue, logit_queue, queue_labels, header,
    )
    nc = _get_program()
    LAST_RESULTS = run_bass_kernel_spmd(nc, in_maps, list(range(N_CORES)))
    parts = np.stack([LAST_RESULTS.results[d]["stats"] for d in range(N_CORES)])
    return combine_stats(parts, M)



# revision 2
# speedup vs baseline: 2.2104x; 2.2104x over previous
"""Trainium2 Bass kernel for nn_CompatibleLearningLoss (MoCo-style queue contrastive loss).

Strategy: shard the queue dimension (Q=32768) across 8 NeuronCores (4096 rows
each).  Each core computes its slice of the three matmuls
    old_embeds  @ feat_queue_shard.T   -> weight
    new_e       @ feat_queue_shard.T   -> scores1
    new_logits  @ logit_queue_shard.T  -> scores2
and reduces per-row partial softmax statistics (chunk max, exp-sum vs chunk
max, masked-weighted raw sums) into a [128, 128] stats tile.  The host combines
the partials in float64 and produces the two scalar losses.

The kernel is HBM-bandwidth bound (the queue matrices are streamed exactly
once), so every matmul operand is quantized to fp8-e4m3 host-side:
  * logit path (dominant stream, was 64 MB/core in bf16 -> 32 MB): both
    operands fp8, matmuls in DoubleRow perf mode (2 MACs/cycle) so TensorE
    (~109 us at 1 col/cycle) stays under the ~105 us DMA roofline.
  * feat path: featT is stored x16 (elements land in e4m3 normal range) while
    the bf16 stationaries (new_e, old_embeds) are stored /16, so the products
    are exact and no device-side rescale is needed.
Marshaling (all host-side): circular queue scatter, new_embeds normalization,
label mask, quantization, and pre-transposition so the contraction dim lands
on SBUF partitions; the device runs pure streaming matmuls.
"""

from contextlib import nullcontext

import numpy as np

import concourse.bass as bass
import concourse.tile as tile
from concourse import mybir
from concourse.bass_utils import run_bass_kernel_spmd
from concourse.vector_clock import ScopedClock

N = 128      # batch
D = 512      # embed dim
C = 8192     # logit dim
Q = 32768    # queue length
N_CORES = 8
QS = Q // N_CORES          # 4096 queue rows per core
EPS = 1e-12
FSCALE = 16.0              # feat-path fp8 scaling (featT x16, stationaries /16)

F32 = mybir.dt.float32
BF16 = mybir.dt.bfloat16
F8 = mybir.dt.float8e4
DR = mybir.MatmulPerfMode.DoubleRow

# stats tile column layout (per core, [128, 128] f32)
# cols 0:8      m1 parts   (feat-path chunk maxes, 8 chunks of 512)
# cols 8:16     z1 parts   (feat-path sum exp(s - chunk max))
# cols 16:24    a1 parts   (feat-path sum maskw * s_raw)
# cols 24:32    W  parts   (sum maskw)
# cols 32:32+B  m2 parts   (logit-path chunk maxes, B_CHUNKS chunks of 512)
# cols 64:64+B  z2 parts
# cols 96:96+B  a2 parts
A_CHUNKS = 8
B_CHUNKS = 8
CB2 = C // 256             # 32 DoubleRow contraction chunks (logit path)


def _split_excess_waits(nc: bass.Bass, limit: int = 1) -> None:
    """This walrus build rejects instructions carrying more than one sync wait
    ("Too many sync wait commands").  Tile's sem-assignment freely attaches
    several.  Move excess waits onto same-engine nops inserted right before
    the offending instruction (queue order makes that equivalent)."""
    for f in nc.m.functions:
        for bb in f.blocks:
            insts = bb.instructions
            insertions = []
            for idx, inst in enumerate(insts):
                si = inst.sync_info
                if si is None:
                    continue
                cap = 2 if isinstance(inst, mybir.InstEventSemaphore) else limit
                waits = list(si.on_wait)
                if len(waits) <= cap:
                    continue
                keep = waits[:cap]
                excess = waits[cap:]
                si.on_wait = keep
                nops = []
                for w in excess:
                    nop = mybir.InstNoOp(
                        name=nc.get_next_instruction_name(), ins=[], outs=[]
                    )
                    nop.engine = inst.engine
                    nop.sync_info = mybir.SyncInfo(on_wait=[w], on_update=[])
                    nc.register_instruction(nop, overwrite=True)
                    nops.append(nop)
                insertions.append((idx, nops))
            for idx, nops in reversed(insertions):
                for nop in reversed(nops):
                    bb.instructions.insert(idx, nop)


class PatchedTileContext(tile.TileContext):
    """Work around the 1-sync-wait-per-instruction cap in this walrus build:
    the stock TileContext tail drain carries one wait per outstanding proc,
    which codegen rejects ("Too many sync wait commands").  Split the waits
    across single-wait SP nops instead."""

    def _drain_and_barrier(self, tick_clock, wait_clock):
        drain_inst = self.nc.sync.drain()
        wait_clock.add_sem_waits(
            drain_inst.ins, ScopedClock({None: tick_clock.global_clock})
        )
        si = drain_inst.ins.sync_info
        if si is not None and len(si.on_wait) > 1:
            waits = list(si.on_wait)
            si.on_wait = [waits[0]]
            for w in waits[1:]:
                nop = self.nc.sync.nop(nofuse=True, hint="drain_wait_split")
                nop.ins.sync_info = mybir.SyncInfo(on_wait=[w], on_update=[])
        self.nc.all_engine_barrier()
        assert self.sems is not None
        popped = self.nc._tile_sem_poison_stack.pop()
        assert popped is self._sem_poison
        self.nc.clear_and_free_semaphores(list(self.sems.allocated().values()))
        self.nc.all_engine_barrier()


def _build_program(repeat: int = 1) -> bass.Bass:
    nc = bass.Bass()

    featT = nc.dram_tensor("featT", [D, QS], F8, kind="ExternalInput")
    logitT = nc.dram_tensor("logitT", [CB2, 128, 2, QS], F8, kind="ExternalInput")
    maskh = nc.dram_tensor("maskh", [N, QS], F8, kind="ExternalInput")
    # stationary operands pre-arranged host-side as their SBUF image
    # [partition, chunk, batch] so each DMA is one contiguous run per partition
    neT = nc.dram_tensor("neT", [128, D // 128, N], BF16, kind="ExternalInput")
    oeT = nc.dram_tensor("oeT", [128, D // 128, N], BF16, kind="ExternalInput")
    nlT = nc.dram_tensor("nlT", [128, C // 128, N], F8, kind="ExternalInput")
    stats = nc.dram_tensor("stats", [N, 128], F32, kind="ExternalOutput")

    AX = mybir.AxisListType
    OP = mybir.AluOpType
    ACT = mybir.ActivationFunctionType

    with PatchedTileContext(nc) as tc:
        with (
            tc.tile_pool(name="const", bufs=1) as const,
            tc.tile_pool(name="small", bufs=4) as small,
            tc.tile_pool(name="scr", bufs=2) as scrp,
            tc.tile_pool(name="ftp", bufs=8) as ftp,
        ):
            # replicated stationary operands, pre-transposed host-side
            neT_sb = const.tile([128, D // 128, N], BF16)
            nc.sync.dma_start(out=neT_sb, in_=neT[:, :, :])
            oeT_sb = const.tile([128, D // 128, N], BF16)
            nc.sync.dma_start(out=oeT_sb, in_=oeT[:, :, :])
            nlT_sb = const.tile([128, C // 128, N], F8)
            nc.sync.dma_start(out=nlT_sb, in_=nlT[:, :, :])

            mw_sb = const.tile([N, QS], F32)
            mh_sb = const.tile([N, QS], F8)
            out_sb = const.tile([N, 128], F32)
            s1c_sb = const.tile([N, A_CHUNKS, 512], F32)  # scores1 parked in SBUF

            def stats_block(src, col_m, col_z, col_a, nch, mw3):
                """Per-row stats over a [128, nch, 512] block `src`:
                chunk maxes -> cols [col_m, col_m+nch), exp-sums vs chunk max
                -> cols [col_z, ...), one total masked-weighted raw sum ->
                col_a.  mw3 is the matching [128, nch, 512] maskw view."""
                nc.vector.tensor_reduce(
                    out=out_sb[:, col_m : col_m + nch], in_=src,
                    axis=AX.X, op=OP.max,
                )
                negm = small.tile([128, 8], F32, tag="negm")
                nc.vector.tensor_scalar_mul(
                    out=negm[:, :nch], in0=out_sb[:, col_m : col_m + nch],
                    scalar1=-1.0,
                )
                for k in range(nch):
                    escr = scrp.tile([128, 512], F32, tag="escr")
                    nc.scalar.activation(
                        out=escr, in_=src[:, k, :], func=ACT.Exp,
                        bias=negm[:, k : k + 1], scale=1.0,
                        accum_out=out_sb[:, col_z + k : col_z + k + 1],
                    )
                # a partial = sum maskw * s_raw (product scratch: s1c_sb)
                nc.vector.scalar_tensor_tensor(
                    out=s1c_sb[:, :nch, :], in0=src, scalar=1.0, in1=mw3,
                    op0=OP.mult, op1=OP.mult,
                    accum_out=out_sb[:, col_a : col_a + 1],
                )

            loop_cm = tc.For_i(0, repeat, 1) if repeat > 1 else nullcontext()
            with loop_cm:
                # 0.5*mask (fp8, exact); maskw lands in f32 mw_sb below
                nc.sync.dma_start(out=mh_sb, in_=maskh[:, :])

                # ---- Phase A: feat path (two q halves of 2048).  Moving
                # operand fp8 (x16), stationaries bf16 (/16) so products are
                # exact.  PSUM results are parked in SBUF immediately
                # (scores1) or consumed by one DVE op (weight -> maskw) so the
                # banks free fast; the ACT-side stats run later, overlapped
                # with phase B's DMA stream.
                with tc.tile_pool(name="psum_a", bufs=1, space="PSUM") as psum_a:
                    for qh in range(2):
                        ps1 = psum_a.tile([128, 4, 512], F32, tag="ps1")
                        psw = psum_a.tile([128, 4, 512], F32, tag="psw")
                        for dc in range(4):
                            ft = ftp.tile([128, 2048], F8, tag="ft")
                            dma_eng = nc.sync if dc % 2 == 0 else nc.scalar
                            dma_eng.dma_start(
                                out=ft,
                                in_=featT[dc * 128 : (dc + 1) * 128,
                                          qh * 2048 : (qh + 1) * 2048],
                            )
                            for qw in range(4):
                                rhs = ft[:, qw * 512 : (qw + 1) * 512]
                                nc.tensor.matmul(
                                    ps1[:, qw, :], neT_sb[:, dc, :], rhs,
                                    start=(dc == 0), stop=(dc == 3),
                                )
                                nc.tensor.matmul(
                                    psw[:, qw, :], oeT_sb[:, dc, :], rhs,
                                    start=(dc == 0), stop=(dc == 3),
                                )
                        hs = slice(qh * 2048, (qh + 1) * 2048)
                        # maskw = (w + 1) * 0.5mask  (fp8 mask in, f32 out)
                        nc.vector.scalar_tensor_tensor(
                            out=mw_sb[:, hs].rearrange("p (c q) -> p c q", c=4),
                            in0=psw, scalar=1.0,
                            in1=mh_sb[:, hs].rearrange("p (c q) -> p c q", c=4),
                            op0=OP.add, op1=OP.mult,
                        )
                        # park scores1 in SBUF so the banks free fast
                        nc.vector.tensor_copy(
                            out=s1c_sb[:, qh * 4 : (qh + 1) * 4, :], in_=ps1
                        )

                # ---- Phase B: logit path (fp8 DoubleRow, 8 psum accumulators)
                with (
                    tc.tile_pool(name="tcp", bufs=8) as tcp,
                    tc.tile_pool(name="psum_b", bufs=1, space="PSUM") as psum_b,
                ):
                    psB = psum_b.tile([128, 8, 512], F32, tag="psB")
                    mw3 = mw_sb.rearrange("p (c q) -> p c q", c=8)

                    def b_step(cb):
                        tcb = tcp.tile([128, 2, QS], F8, tag="tcb", name="tcb")
                        dma_eng = nc.sync if cb % 2 == 0 else nc.scalar
                        dma_eng.dma_start(out=tcb, in_=logitT[cb, :, :, :])
                        for qw in range(8):
                            nc.tensor.matmul(
                                psB[:, qw, :],
                                nlT_sb[:, 2 * cb : 2 * cb + 2, :],
                                tcb[:, :, qw * 512 : (qw + 1) * 512],
                                start=(cb == 0), stop=(cb == CB2 - 1),
                                perf_mode=DR,
                            )

                    for cb in range(12):
                        b_step(cb)
                    # deferred phase A stats, interleaved mid-stream so the
                    # DVE/ACT work hides under the logit DMA stream
                    nc.vector.tensor_reduce(
                        out=out_sb[:, 24:32], in_=mw3, axis=AX.X, op=OP.add,
                    )
                    stats_block(s1c_sb, 0, 8, 16, A_CHUNKS, mw3)
                    for cb in range(12, CB2):
                        b_step(cb)
                    # phase B stats
                    stats_block(psB, 32, 64, 96, 8, mw3)

            nc.sync.dma_start(out=stats[:, :], in_=out_sb)

    _split_excess_waits(nc)
    return nc


_PROGRAM: bass.Bass | None = None
LAST_RESULTS = None  # BassKernelResults of the most recent run (for profiling)


def _get_program() -> bass.Bass:
    global _PROGRAM
    if _PROGRAM is None:
        _PROGRAM = _build_program()
    return _PROGRAM


def _to_f8(t):
    """torch f32 tensor -> numpy ml_dtypes.float8_e4m3 view, same shape.
    torch's e4m3fn and TRN/ml_dtypes e4m3 agree bit-for-bit for |x| <= 240;
    all tensors quantized here are well inside that."""
    import ml_dtypes
    import torch

    return (
        t.to(torch.float8_e4m3fn).view(torch.int8).numpy()
        .view(ml_dtypes.float8_e4m3)
    )


def host_prep(old_embeds, old_logits, new_embeds, new_logits, labels,
              feat_queue, logit_queue, queue_labels, header):
    """Scatter + normalize + mask + quantize + pre-transpose on host; returns
    per-core in_maps and the per-row positive counts M."""
    import ml_dtypes
    import torch

    old_embeds = np.asarray(old_embeds, dtype=np.float32)
    old_logits = np.asarray(old_logits, dtype=np.float32)
    new_embeds = np.asarray(new_embeds, dtype=np.float32)
    new_logits = np.asarray(new_logits, dtype=np.float32)
    feat_queue = np.array(feat_queue, dtype=np.float32)   # copies (scattered below)
    logit_queue = np.array(logit_queue, dtype=np.float32)
    labels_np = np.asarray(labels).astype(np.int64)
    queue_labels_np = np.asarray(queue_labels).astype(np.int64)
    hdr = int(np.asarray(header))

    n = old_embeds.shape[0]
    q = feat_queue.shape[0]
    assert (n, q) == (N, Q)

    # circular queue scatter
    idx = (hdr + np.arange(n)) % q
    feat_queue[idx] = old_embeds
    logit_queue[idx] = old_logits
    queue_labels_np[idx] = labels_np

    # normalize new_embeds (f64 intermediate, f32 result)
    ne64 = new_embeds.astype(np.float64)
    norm = np.sqrt((ne64 * ne64).sum(axis=1, keepdims=True))
    new_e = (ne64 / np.maximum(norm, EPS)).astype(np.float32)

    # label mask (host): maskh = 0.5 * mask (fp8: 0.5 is exact)
    mask = (queue_labels_np[None, :] == labels_np[:, None])
    M = mask.sum(axis=1).astype(np.float64)               # [N], >= 1 by construction
    maskh = (0.5 * mask.astype(np.float32)).astype(ml_dtypes.float8_e4m3)

    def _sbuf_image(aT):
        # [K, N] -> [128, K//128, N] partition-major SBUF image
        k = aT.shape[0]
        return np.ascontiguousarray(
            aT.reshape(k // 128, 128, aT.shape[1]).transpose(1, 0, 2)
        )

    neT = _sbuf_image(np.ascontiguousarray((new_e / FSCALE).T)).astype(
        ml_dtypes.bfloat16)                                   # [128, 4, N] bf16
    oeT = _sbuf_image(np.ascontiguousarray((old_embeds / FSCALE).T)).astype(
        ml_dtypes.bfloat16)                                   # [128, 4, N] bf16
    nlT_t = torch.from_numpy(new_logits).t().contiguous()     # [C, N]
    nlT = _sbuf_image(_to_f8(nlT_t))                          # [128, 64, N] fp8

    # fp8 queue shards.  featT is scaled x16 so elements (~N(0, 1/512)) land
    # in e4m3's normal range; the bf16 stationaries above carry the /16.
    fq_t = torch.from_numpy(feat_queue)
    lq8 = torch.from_numpy(logit_queue).to(torch.float8_e4m3fn)  # [Q, C]

    in_maps = []
    for d in range(N_CORES):
        sl = slice(d * QS, (d + 1) * QS)
        featT = _to_f8((fq_t[sl].t() * FSCALE).contiguous())     # [D, QS] fp8
        # logit shard -> DoubleRow moving layout [CB2, 128, 2, QS]:
        # [c2, p, r, j] = logit_queue[qs0 + j, c2*256 + r*128 + p]
        lsh = lq8[sl].view(torch.int8).t().contiguous()          # [C, QS] i8
        ldr = (lsh.reshape(CB2, 2, 128, QS).permute(0, 2, 1, 3)
               .contiguous().numpy().view(ml_dtypes.float8_e4m3))
        in_maps.append({
            "featT": featT,
            "logitT": ldr,
            "maskh": np.ascontiguousarray(maskh[:, sl]),
            "neT": neT,
            "oeT": oeT,
            "nlT": nlT,
        })
    return in_maps, M


def combine_stats(parts: np.ndarray, M: np.ndarray):
    """parts: [n_cores, 128, 128] f32 stats tiles -> (l1, l2) f32 scalars."""
    parts = parts.astype(np.float64)
    m1p = parts[:, :, 0:8]
    z1p = parts[:, :, 8:16]
    a1p = parts[:, :, 16:17]
    wp = parts[:, :, 24:32]
    m2p = parts[:, :, 32 : 32 + B_CHUNKS]
    z2p = parts[:, :, 64 : 64 + B_CHUNKS]
    a2p = parts[:, :, 96:97]

    W = wp.sum(axis=(0, 2))                               # [N]
    A1 = a1p.sum(axis=(0, 2))
    A2 = a2p.sum(axis=(0, 2))
    m1 = m1p.max(axis=(0, 2))
    m2 = m2p.max(axis=(0, 2))
    Z1 = (z1p * np.exp(m1p - m1[None, :, None])).sum(axis=(0, 2))
    Z2 = (z2p * np.exp(m2p - m2[None, :, None])).sum(axis=(0, 2))

    # sum_j maskw * log_prob = A_raw - (m + log Z) * W ; divide by count, mean, negate
    l1 = -np.mean((A1 - (m1 + np.log(Z1)) * W) / M)
    l2 = -np.mean((A2 - (m2 + np.log(Z2)) * W) / M)
    return (np.float32(l1), np.float32(l2))


def kernel(old_embeds, old_logits, new_embeds, new_logits, labels,
           feat_queue, logit_queue, queue_labels, header):
    global LAST_RESULTS
    in_maps, M = host_prep(
        old_embeds, old_logits, new_embeds, new_logits, labels,
        feat_queue, logit_queue, queue_labels, header,
    )
    nc = _get_program()
    LAST_RESULTS = run_bass_kernel_spmd(nc, in_maps, list(range(N_CORES)))
    parts = np.stack([LAST_RESULTS.results[d]["stats"] for d in range(N_CORES)])
    return combine_stats(parts, M)


# revision 15
# speedup vs baseline: 2.2595x; 1.0222x over previous
"""Trainium2 Bass kernel for nn_CompatibleLearningLoss (MoCo-style queue contrastive loss).

Strategy: shard the queue dimension (Q=32768) across 8 NeuronCores (4096 rows
each).  Each core computes its slice of the three matmuls
    old_embeds  @ feat_queue_shard.T   -> weight
    new_e       @ feat_queue_shard.T   -> scores1
    new_logits  @ logit_queue_shard.T  -> scores2
and reduces per-row partial softmax statistics (chunk max, exp-sum vs chunk
max, masked-weighted raw sums) into a [128, 128] stats tile.  The host combines
the partials in float64 and produces the two scalar losses.

The kernel is jointly HBM- and TensorE-bound (~50 us each per probe), so:
  * every matmul operand is fp8-e4m3 (logit path: both sides fp8, DoubleRow
    perf mode = 2 MACs/cycle; feat path: featT stored x16 so elements land in
    e4m3 normal range, bf16 stationaries store the /16, products exact);
  * the logit stream is split into two column halves with separate 4-bank
    PSUM accumulators so the first half's softmax stats overlap the second
    half's DMA/matmul stream (short tail);
  * phase A (feat) runs in four 1024-column quarters ping-ponging 2+2 PSUM
    banks so TensorE never waits on the DVE consumers;
  * bulk logit DMA owns the sync+scalar rings; all small tensors ride the
    vector ring.
Marshaling (all host-side): circular queue scatter, new_embeds normalization,
label mask, quantization, and pre-transposition so the contraction dim lands
on SBUF partitions; the device runs pure streaming matmuls.
"""

from contextlib import nullcontext

import numpy as np

import concourse.bass as bass
import concourse.tile as tile
from concourse import mybir
from concourse.bass_utils import run_bass_kernel_spmd
from concourse.vector_clock import ScopedClock

N = 128      # batch
D = 512      # embed dim
C = 8192     # logit dim
Q = 32768    # queue length
N_CORES = 8
QS = Q // N_CORES          # 4096 queue rows per core
QH = QS // 2               # 2048-column logit half
EPS = 1e-12
FSCALE = 16.0              # feat-path fp8 scaling (featT x16, stationaries /16)

F32 = mybir.dt.float32
BF16 = mybir.dt.bfloat16
F8 = mybir.dt.float8e4
DR = mybir.MatmulPerfMode.DoubleRow

# stats tile column layout (per core, [128, 128] f32)
# cols 0:8      m1 parts   (feat-path chunk maxes, 8 chunks of 512)
# cols 8:16     z1 parts   (feat-path sum exp(s - chunk max))
# cols 16:17    a1 part    (feat-path sum maskw * s_raw)
# cols 24:32    W  parts   (sum maskw)
# cols 32:40    m2 parts   (logit-path chunk maxes, halves at 32 / 36)
# cols 64:72    z2 parts   (halves at 64 / 68)
# cols 96:98    a2 parts   (halves at 96 / 97)
A_CHUNKS = 8
CB2 = C // 256             # 32 DoubleRow contraction chunks (logit path)


def _split_excess_waits(nc: bass.Bass, limit: int = 1) -> None:
    """This walrus build rejects instructions carrying more than one sync wait
    ("Too many sync wait commands").  Tile's sem-assignment freely attaches
    several.  Move excess waits onto same-engine nops inserted right before
    the offending instruction (queue order makes that equivalent)."""
    for f in nc.m.functions:
        for bb in f.blocks:
            insts = bb.instructions
            insertions = []
            for idx, inst in enumerate(insts):
                si = inst.sync_info
                if si is None:
                    continue
                cap = 2 if isinstance(inst, mybir.InstEventSemaphore) else limit
                waits = list(si.on_wait)
                if len(waits) <= cap:
                    continue
                keep = waits[:cap]
                excess = waits[cap:]
                si.on_wait = keep
                nops = []
                for w in excess:
                    nop = mybir.InstNoOp(
                        name=nc.get_next_instruction_name(), ins=[], outs=[]
                    )
                    nop.engine = inst.engine
                    nop.sync_info = mybir.SyncInfo(on_wait=[w], on_update=[])
                    nc.register_instruction(nop, overwrite=True)
                    nops.append(nop)
                insertions.append((idx, nops))
            for idx, nops in reversed(insertions):
                for nop in reversed(nops):
                    bb.instructions.insert(idx, nop)


class PatchedTileContext(tile.TileContext):
    """Work around the 1-sync-wait-per-instruction cap in this walrus build:
    the stock TileContext tail drain carries one wait per outstanding proc,
    which codegen rejects ("Too many sync wait commands").  Split the waits
    across single-wait SP nops instead."""

    def _drain_and_barrier(self, tick_clock, wait_clock):
        drain_inst = self.nc.sync.drain()
        wait_clock.add_sem_waits(
            drain_inst.ins, ScopedClock({None: tick_clock.global_clock})
        )
        si = drain_inst.ins.sync_info
        if si is not None and len(si.on_wait) > 1:
            waits = list(si.on_wait)
            si.on_wait = [waits[0]]
            for w in waits[1:]:
                nop = self.nc.sync.nop(nofuse=True, hint="drain_wait_split")
                nop.ins.sync_info = mybir.SyncInfo(on_wait=[w], on_update=[])
        self.nc.all_engine_barrier()
        assert self.sems is not None
        popped = self.nc._tile_sem_poison_stack.pop()
        assert popped is self._sem_poison
        self.nc.clear_and_free_semaphores(list(self.sems.allocated().values()))
        self.nc.all_engine_barrier()


def _build_program(repeat: int = 1, mode: str = "full") -> bass.Bass:
    # mode: "full" | "dma" (skip matmuls+stats) | "mm" (skip DMAs) — perf probes
    do_mm = mode != "dma"
    do_dma = mode != "mm"
    do_stats = mode == "full"
    nc = bass.Bass()

    featT = nc.dram_tensor("featT", [D, QS], F8, kind="ExternalInput")
    # DoubleRow moving layout, split into two 2048-column halves:
    # [h, c2, p, r, j] = logit_queue[qs0 + h*QH + j, c2*256 + r*128 + p]
    logitT = nc.dram_tensor("logitT", [2, CB2, 128, 2, QH], F8,
                            kind="ExternalInput")
    maskh = nc.dram_tensor("maskh", [N, QS], F8, kind="ExternalInput")
    # stationary operands pre-arranged host-side as their SBUF image
    # [partition, chunk, batch] so each DMA is one contiguous run per partition
    neT = nc.dram_tensor("neT", [128, D // 128, N], BF16, kind="ExternalInput")
    oeT = nc.dram_tensor("oeT", [128, D // 128, N], BF16, kind="ExternalInput")
    nlT = nc.dram_tensor("nlT", [128, C // 128, N], F8, kind="ExternalInput")
    stats = nc.dram_tensor("stats", [N, 128], F32, kind="ExternalOutput")

    AX = mybir.AxisListType
    OP = mybir.AluOpType
    ACT = mybir.ActivationFunctionType

    with PatchedTileContext(nc) as tc:
        with (
            tc.tile_pool(name="const", bufs=1) as const,
            tc.tile_pool(name="small", bufs=4) as small,
            tc.tile_pool(name="scr", bufs=2) as scrp,
            tc.tile_pool(name="ftp", bufs=8) as ftp,
        ):
            # replicated stationary operands, pre-transposed host-side
            neT_sb = const.tile([128, D // 128, N], BF16)
            nc.gpsimd.dma_start(out=neT_sb, in_=neT[:, :, :])
            oeT_sb = const.tile([128, D // 128, N], BF16)
            nc.gpsimd.dma_start(out=oeT_sb, in_=oeT[:, :, :])
            nlT_sb = const.tile([128, C // 128, N], F8)
            nc.gpsimd.dma_start(out=nlT_sb, in_=nlT[:, :, :])

            mw_sb = const.tile([N, QS], BF16)
            mh_sb = const.tile([N, QS], F8)
            out_sb = const.tile([N, 128], F32)
            s1c_sb = const.tile([N, A_CHUNKS, 512], BF16)  # scores1 parked in SBUF
            if not do_stats:
                nc.vector.memset(out_sb, 0.0)

            def stats_block(src, col_m, col_z, col_a, nch, mw3, scratch):
                """Per-row stats over a [128, nch, 512] block `src`:
                chunk maxes -> cols [col_m, col_m+nch), exp-sums vs chunk max
                -> cols [col_z, ...), one total masked-weighted raw sum ->
                col_a.  mw3 is the matching [128, nch, 512] maskw view."""
                nc.vector.tensor_reduce(
                    out=out_sb[:, col_m : col_m + nch], in_=src,
                    axis=AX.X, op=OP.max,
                )
                negm = small.tile([128, 8], F32, tag="negm")
                nc.vector.tensor_scalar_mul(
                    out=negm[:, :nch], in0=out_sb[:, col_m : col_m + nch],
                    scalar1=-1.0,
                )
                for k in range(nch):
                    escr = scrp.tile([128, 512], F32, tag="escr")
                    nc.scalar.activation(
                        out=escr, in_=src[:, k, :], func=ACT.Exp,
                        bias=negm[:, k : k + 1], scale=1.0,
                        accum_out=out_sb[:, col_z + k : col_z + k + 1],
                    )
                # a partial = sum maskw * s_raw
                nc.vector.scalar_tensor_tensor(
                    out=scratch[:, :nch, :], in0=src, scalar=1.0, in1=mw3,
                    op0=OP.mult, op1=OP.mult,
                    accum_out=out_sb[:, col_a : col_a + 1],
                )

            loop_cm = tc.For_i(0, repeat, 1) if repeat > 1 else nullcontext()
            with loop_cm:
                # 0.5*mask (fp8, exact); maskw lands in bf16 mw_sb below
                if do_dma:
                    nc.sync.dma_start(out=mh_sb, in_=maskh[:, :])

                # ---- Phase A: feat path, four 1024-column quarters.  Moving
                # operand fp8 (x16), stationaries bf16 (/16) so products are
                # exact.  2+2 PSUM banks per quarter, pool bufs=2: quarter
                # k+1's matmuls run while quarter k's PSUM is drained by the
                # DVE (maskw merge + scores1 parking).
                with tc.tile_pool(name="psum_a", bufs=2, space="PSUM") as psum_a:
                    for qq in range(4):
                        ps1 = psum_a.tile([128, 2, 512], F32, tag="ps1")
                        psw = psum_a.tile([128, 2, 512], F32, tag="psw")
                        for dc in range(4):
                            ft = ftp.tile([128, 1024], F8, tag="ft")
                            dma_eng = nc.sync if dc % 2 == 0 else nc.scalar
                            if do_dma:
                                dma_eng.dma_start(
                                    out=ft,
                                    in_=featT[dc * 128 : (dc + 1) * 128,
                                              qq * 1024 : (qq + 1) * 1024],
                                )
                            elif do_mm:
                                dma_eng.dma_start(
                                    out=ft[:, 0:16],
                                    in_=featT[dc * 128 : (dc + 1) * 128, 0:16],
                                )
                            for qw in range(2):
                                if not do_mm:
                                    break
                                rhs = ft[:, qw * 512 : (qw + 1) * 512]
                                nc.tensor.matmul(
                                    ps1[:, qw, :], neT_sb[:, dc, :], rhs,
                                    start=(dc == 0), stop=(dc == 3),
                                )
                                nc.tensor.matmul(
                                    psw[:, qw, :], oeT_sb[:, dc, :], rhs,
                                    start=(dc == 0), stop=(dc == 3),
                                )
                        hs = slice(qq * 1024, (qq + 1) * 1024)
                        if do_stats:
                            # maskw = (w + 1) * 0.5mask  (fp8 mask in, bf16 out)
                            nc.vector.scalar_tensor_tensor(
                                out=mw_sb[:, hs].rearrange("p (c q) -> p c q", c=2),
                                in0=psw, scalar=1.0,
                                in1=mh_sb[:, hs].rearrange("p (c q) -> p c q", c=2),
                                op0=OP.add, op1=OP.mult,
                            )
                            # park scores1 in SBUF so the banks free fast
                            nc.vector.tensor_copy(
                                out=s1c_sb[:, qq * 2 : (qq + 1) * 2, :], in_=ps1
                            )

                # ---- Phase B: logit path (fp8 DoubleRow), two column halves
                # with separate 4-bank accumulators; half 0's stats run under
                # half 1's DMA/matmul stream.
                with (
                    tc.tile_pool(name="tcp", bufs=14) as tcp,
                    tc.tile_pool(name="psum_b", bufs=1, space="PSUM") as psum_b,
                ):
                    psB = [psum_b.tile([128, 4, 512], F32, tag=f"psB{h}",
                                       name=f"psB{h}")
                           for h in range(2)]
                    mw3 = [
                        mw_sb[:, h * QH : (h + 1) * QH]
                        .rearrange("p (c q) -> p c q", c=4)
                        for h in range(2)
                    ]

                    def b_step(h, cb):
                        tcb = tcp.tile([128, 2, QH], F8, tag="tcb", name="tcb")
                        dma_eng = nc.sync if cb % 2 == 0 else nc.scalar
                        if do_dma:
                            dma_eng.dma_start(out=tcb, in_=logitT[h, cb, :, :, :])
                        elif do_mm:
                            dma_eng.dma_start(
                                out=tcb[:, :, 0:16], in_=logitT[h, cb, :, :, 0:16]
                            )
                        for qw in range(4):
                            if not do_mm:
                                break
                            nc.tensor.matmul(
                                psB[h][:, qw, :],
                                nlT_sb[:, 2 * cb : 2 * cb + 2, :],
                                tcb[:, :, qw * 512 : (qw + 1) * 512],
                                start=(cb == 0), stop=(cb == CB2 - 1),
                                perf_mode=DR,
                            )

                    for cb in range(12):
                        b_step(0, cb)
                    if do_stats:
                        # deferred phase A stats, interleaved mid-stream so the
                        # DVE/ACT work hides under the logit DMA stream
                        nc.vector.tensor_reduce(
                            out=out_sb[:, 24:32],
                            in_=mw_sb.rearrange("p (c q) -> p c q", c=8),
                            axis=AX.X, op=OP.add,
                        )
                        stats_block(s1c_sb, 0, 8, 16, A_CHUNKS,
                                    mw_sb.rearrange("p (c q) -> p c q", c=8),
                                    s1c_sb)
                    for cb in range(12, CB2):
                        b_step(0, cb)
                    for cb in range(6):
                        b_step(1, cb)
                    if do_stats:
                        # half-0 stats while half 1 streams
                        stats_block(psB[0], 32, 64, 96, 4, mw3[0], s1c_sb)
                    for cb in range(6, CB2):
                        b_step(1, cb)
                    if do_stats:
                        stats_block(psB[1], 36, 68, 97, 4, mw3[1], s1c_sb)

            nc.sync.dma_start(out=stats[:, :], in_=out_sb)

    _split_excess_waits(nc)
    return nc


_PROGRAM: bass.Bass | None = None
LAST_RESULTS = None  # BassKernelResults of the most recent run (for profiling)


def _get_program() -> bass.Bass:
    global _PROGRAM
    if _PROGRAM is None:
        _PROGRAM = _build_program()
    return _PROGRAM


def _to_f8(t):
    """torch f32 tensor -> numpy ml_dtypes.float8_e4m3 view, same shape.
    torch's e4m3fn and TRN/ml_dtypes e4m3 agree bit-for-bit for |x| <= 240;
    all tensors quantized here are well inside that."""
    import ml_dtypes
    import torch

    return (
        t.to(torch.float8_e4m3fn).view(torch.int8).numpy()
        .view(ml_dtypes.float8_e4m3)
    )


def host_prep(old_embeds, old_logits, new_embeds, new_logits, labels,
              feat_queue, logit_queue, queue_labels, header):
    """Scatter + normalize + mask + quantize + pre-transpose on host; returns
    per-core in_maps and the per-row positive counts M."""
    import ml_dtypes
    import torch

    old_embeds = np.asarray(old_embeds, dtype=np.float32)
    old_logits = np.asarray(old_logits, dtype=np.float32)
    new_embeds = np.asarray(new_embeds, dtype=np.float32)
    new_logits = np.asarray(new_logits, dtype=np.float32)
    feat_queue = np.array(feat_queue, dtype=np.float32)   # copies (scattered below)
    logit_queue = np.array(logit_queue, dtype=np.float32)
    labels_np = np.asarray(labels).astype(np.int64)
    queue_labels_np = np.asarray(queue_labels).astype(np.int64)
    hdr = int(np.asarray(header))

    n = old_embeds.shape[0]
    q = feat_queue.shape[0]
    assert (n, q) == (N, Q)

    # circular queue scatter
    idx = (hdr + np.arange(n)) % q
    feat_queue[idx] = old_embeds
    logit_queue[idx] = old_logits
    queue_labels_np[idx] = labels_np

    # normalize new_embeds (f64 intermediate, f32 result)
    ne64 = new_embeds.astype(np.float64)
    norm = np.sqrt((ne64 * ne64).sum(axis=1, keepdims=True))
    new_e = (ne64 / np.maximum(norm, EPS)).astype(np.float32)

    # label mask (host): maskh = 0.5 * mask (fp8: 0.5 is exact)
    mask = (queue_labels_np[None, :] == labels_np[:, None])
    M = mask.sum(axis=1).astype(np.float64)               # [N], >= 1 by construction
    maskh = (0.5 * mask.astype(np.float32)).astype(ml_dtypes.float8_e4m3)

    def _sbuf_image(aT):
        # [K, N] -> [128, K//128, N] partition-major SBUF image
        k = aT.shape[0]
        return np.ascontiguousarray(
            aT.reshape(k // 128, 128, aT.shape[1]).transpose(1, 0, 2)
        )

    neT = _sbuf_image(np.ascontiguousarray((new_e / FSCALE).T)).astype(
        ml_dtypes.bfloat16)                                   # [128, 4, N] bf16
    oeT = _sbuf_image(np.ascontiguousarray((old_embeds / FSCALE).T)).astype(
        ml_dtypes.bfloat16)                                   # [128, 4, N] bf16
    nlT_t = torch.from_numpy(new_logits).t().contiguous()     # [C, N]
    nlT = _sbuf_image(_to_f8(nlT_t))                          # [128, 64, N] fp8

    # fp8 queue shards.  featT is scaled x16 so elements (~N(0, 1/512)) land
    # in e4m3's normal range; the bf16 stationaries above carry the /16.
    fq_t = torch.from_numpy(feat_queue)
    lq8 = torch.from_numpy(logit_queue).to(torch.float8_e4m3fn)  # [Q, C]

    in_maps = []
    for d in range(N_CORES):
        sl = slice(d * QS, (d + 1) * QS)
        featT = _to_f8((fq_t[sl].t() * FSCALE).contiguous())     # [D, QS] fp8
        # logit shard -> DoubleRow moving layout [2, CB2, 128, 2, QH]:
        # [h, c2, p, r, j] = logit_queue[qs0 + h*QH + j, c2*256 + r*128 + p]
        lsh = lq8[sl].view(torch.int8).t().contiguous()          # [C, QS] i8
        ldr = (lsh.reshape(CB2, 2, 128, 2, QH).permute(3, 0, 2, 1, 4)
               .contiguous().numpy().view(ml_dtypes.float8_e4m3))
        in_maps.append({
            "featT": featT,
            "logitT": ldr,
            "maskh": np.ascontiguousarray(maskh[:, sl]),
            "neT": neT,
            "oeT": oeT,
            "nlT": nlT,
        })
    return in_maps, M


def combine_stats(parts: np.ndarray, M: np.ndarray):
    """parts: [n_cores, 128, 128] f32 stats tiles -> (l1, l2) f32 scalars."""
    parts = parts.astype(np.float64)
    m1p = parts[:, :, 0:8]
    z1p = parts[:, :, 8:16]
    a1p = parts[:, :, 16:17]
    wp = parts[:, :, 24:32]
    m2p = parts[:, :, 32:40]
    z2p = parts[:, :, 64:72]
    a2p = parts[:, :, 96:98]

    W = wp.sum(axis=(0, 2))                               # [N]
    A1 = a1p.sum(axis=(0, 2))
    A2 = a2p.sum(axis=(0, 2))
    m1 = m1p.max(axis=(0, 2))
    m2 = m2p.max(axis=(0, 2))
    Z1 = (z1p * np.exp(m1p - m1[None, :, None])).sum(axis=(0, 2))
    Z2 = (z2p * np.exp(m2p - m2[None, :, None])).sum(axis=(0, 2))

    # sum_j maskw * log_prob = A_raw - (m + log Z) * W ; divide by count, mean, negate
    l1 = -np.mean((A1 - (m1 + np.log(Z1)) * W) / M)
    l2 = -np.mean((A2 - (m2 + np.log(Z2)) * W) / M)
    return (np.float32(l1), np.float32(l2))


def kernel(old_embeds, old_logits, new_embeds, new_logits, labels,
           feat_queue, logit_queue, queue_labels, header):
    global LAST_RESULTS
    in_maps, M = host_prep(
        old_embeds, old_logits, new_embeds, new_logits, labels,
        feat_queue, logit_queue, queue_labels, header,
    )
    nc = _get_program()
    LAST_RESULTS = run_bass_kernel_spmd(nc, in_maps, list(range(N_CORES)))
    parts = np.stack([LAST_RESULTS.results[d]["stats"] for d in range(N_CORES)])
    return combine_stats(parts, M)


# revision 17
# speedup vs baseline: 2.7646x; 1.2236x over previous
"""Trainium2 Bass kernel for nn_CompatibleLearningLoss (MoCo-style queue contrastive loss).

Splits the loss  -mean_i( sum_j mask*w*(s_j - m - logZ) / M_i )  into
  * a sparse part  A = sum_j mask*w*s_j  and  W = sum_j mask*w : the label
    mask has ~Q/C = 4 positives per row, so these are ~512 short dot products
    — computed EXACTLY on the host in float64;
  * a dense part  m_i = max_j s_j  and  Z_i = sum_j exp(s_j - m_i)  over the
    full queue, for both score matrices
        scores1 = new_e      @ feat_queue.T      [N, Q]
        scores2 = new_logits @ logit_queue.T     [N, Q]
    — the only work that actually needs the 1 GB queue streamed, done on
    device with the queue dimension sharded across 8 NeuronCores.

Each core streams its 4096-row queue shard once and emits per-row partial
softmax stats (chunk maxes + exp-sums vs chunk max) into a [128, 128] stats
tile; the host combines partials in float64.

The kernel is jointly HBM- and TensorE-bound, so every matmul operand is
fp8-e4m3: the logit path (33.5 MB/core stream) runs both sides fp8 with
DoubleRow perf mode (2 MACs/cycle); the feat path stores featT x16 so
elements land in e4m3's normal range with bf16 stationaries carrying the /16
(products exact).  The logit stream is split into two column halves with
separate 4-bank PSUM accumulators so the first half's stats overlap the
second half's DMA/matmul stream, and phase A (feat) runs in four 1024-column
quarters ping-ponging 2-bank PSUM tiles so TensorE never waits on the DVE.
Marshaling (all host-side): circular queue scatter, new_embeds normalization,
sparse-part evaluation, fp8 quantization, and pre-transposition so the
contraction dim lands on SBUF partitions.
"""

from contextlib import nullcontext

import numpy as np

import concourse.bass as bass
import concourse.tile as tile
from concourse import mybir
from concourse.bass_utils import run_bass_kernel_spmd
from concourse.vector_clock import ScopedClock

N = 128      # batch
D = 512      # embed dim
C = 8192     # logit dim
Q = 32768    # queue length
N_CORES = 8
QS = Q // N_CORES          # 4096 queue rows per core
QH = QS // 2               # 2048-column logit half
EPS = 1e-12
FSCALE = 16.0              # feat-path fp8 scaling (featT x16, stationaries /16)

F32 = mybir.dt.float32
BF16 = mybir.dt.bfloat16
F8 = mybir.dt.float8e4
DR = mybir.MatmulPerfMode.DoubleRow

# stats tile column layout (per core, [128, 128] f32)
# cols 0:8      m1 parts   (feat-path chunk maxes, 8 chunks of 512)
# cols 8:16     z1 parts   (feat-path sum exp(s - chunk max))
# cols 32:40    m2 parts   (logit-path chunk maxes, halves at 32 / 36)
# cols 64:72    z2 parts   (halves at 64 / 68)
A_CHUNKS = 8
CB2 = C // 256             # 32 DoubleRow contraction chunks (logit path)


def _split_excess_waits(nc: bass.Bass, limit: int = 1) -> None:
    """This walrus build rejects instructions carrying more than one sync wait
    ("Too many sync wait commands").  Tile's sem-assignment freely attaches
    several.  Move excess waits onto same-engine nops inserted right before
    the offending instruction (queue order makes that equivalent)."""
    for f in nc.m.functions:
        for bb in f.blocks:
            insts = bb.instructions
            insertions = []
            for idx, inst in enumerate(insts):
                si = inst.sync_info
                if si is None:
                    continue
                cap = 2 if isinstance(inst, mybir.InstEventSemaphore) else limit
                waits = list(si.on_wait)
                if len(waits) <= cap:
                    continue
                keep = waits[:cap]
                excess = waits[cap:]
                si.on_wait = keep
                nops = []
                for w in excess:
                    nop = mybir.InstNoOp(
                        name=nc.get_next_instruction_name(), ins=[], outs=[]
                    )
                    nop.engine = inst.engine
                    nop.sync_info = mybir.SyncInfo(on_wait=[w], on_update=[])
                    nc.register_instruction(nop, overwrite=True)
                    nops.append(nop)
                insertions.append((idx, nops))
            for idx, nops in reversed(insertions):
                for nop in reversed(nops):
                    bb.instructions.insert(idx, nop)


class PatchedTileContext(tile.TileContext):
    """Work around the 1-sync-wait-per-instruction cap in this walrus build:
    the stock TileContext tail drain carries one wait per outstanding proc,
    which codegen rejects ("Too many sync wait commands").  Split the waits
    across single-wait SP nops instead."""

    def _drain_and_barrier(self, tick_clock, wait_clock):
        drain_inst = self.nc.sync.drain()
        wait_clock.add_sem_waits(
            drain_inst.ins, ScopedClock({None: tick_clock.global_clock})
        )
        si = drain_inst.ins.sync_info
        if si is not None and len(si.on_wait) > 1:
            waits = list(si.on_wait)
            si.on_wait = [waits[0]]
            for w in waits[1:]:
                nop = self.nc.sync.nop(nofuse=True, hint="drain_wait_split")
                nop.ins.sync_info = mybir.SyncInfo(on_wait=[w], on_update=[])
        self.nc.all_engine_barrier()
        assert self.sems is not None
        popped = self.nc._tile_sem_poison_stack.pop()
        assert popped is self._sem_poison
        self.nc.clear_and_free_semaphores(list(self.sems.allocated().values()))
        self.nc.all_engine_barrier()


def _build_program(repeat: int = 1, mode: str = "full") -> bass.Bass:
    # mode: "full" | "dma" (skip matmuls+stats) | "mm" (skip DMAs) — perf probes
    do_mm = mode != "dma"
    do_dma = mode != "mm"
    do_stats = mode == "full"
    nc = bass.Bass()

    featT = nc.dram_tensor("featT", [D, QS], F8, kind="ExternalInput")
    # DoubleRow moving layout, split into two 2048-column halves:
    # [h, c2, p, r, j] = logit_queue[qs0 + h*QH + j, c2*256 + r*128 + p]
    logitT = nc.dram_tensor("logitT", [2, CB2, 128, 2, QH], F8,
                            kind="ExternalInput")
    # stationary operands pre-arranged host-side as their SBUF image
    # [partition, chunk, batch] so each DMA is one contiguous run per partition
    neT = nc.dram_tensor("neT", [128, D // 128, N], BF16, kind="ExternalInput")
    nlT = nc.dram_tensor("nlT", [128, C // 128, N], F8, kind="ExternalInput")
    stats = nc.dram_tensor("stats", [N, 128], F32, kind="ExternalOutput")

    AX = mybir.AxisListType
    OP = mybir.AluOpType
    ACT = mybir.ActivationFunctionType

    with PatchedTileContext(nc) as tc:
        with (
            tc.tile_pool(name="const", bufs=1) as const,
            tc.tile_pool(name="small", bufs=4) as small,
            tc.tile_pool(name="scr", bufs=2) as scrp,
            tc.tile_pool(name="ftp", bufs=8) as ftp,
        ):
            # replicated stationary operands, pre-transposed host-side
            neT_sb = const.tile([128, D // 128, N], BF16)
            nc.gpsimd.dma_start(out=neT_sb, in_=neT[:, :, :])
            nlT_sb = const.tile([128, C // 128, N], F8)
            nc.gpsimd.dma_start(out=nlT_sb, in_=nlT[:, :, :])

            out_sb = const.tile([N, 128], F32)
            s1c_sb = const.tile([N, A_CHUNKS, 512], BF16)  # scores1 parked in SBUF
            if not do_stats:
                nc.vector.memset(out_sb, 0.0)

            def stats_block(src, col_m, col_z, nch):
                """Per-row softmax stats over a [128, nch, 512] block `src`:
                chunk maxes -> cols [col_m, col_m+nch), exp-sums vs own chunk
                max -> cols [col_z, col_z+nch)."""
                nc.vector.tensor_reduce(
                    out=out_sb[:, col_m : col_m + nch], in_=src,
                    axis=AX.X, op=OP.max,
                )
                negm = small.tile([128, 8], F32, tag="negm")
                nc.vector.tensor_scalar_mul(
                    out=negm[:, :nch], in0=out_sb[:, col_m : col_m + nch],
                    scalar1=-1.0,
                )
                for k in range(nch):
                    escr = scrp.tile([128, 512], F32, tag="escr")
                    nc.scalar.activation(
                        out=escr, in_=src[:, k, :], func=ACT.Exp,
                        bias=negm[:, k : k + 1], scale=1.0,
                        accum_out=out_sb[:, col_z + k : col_z + k + 1],
                    )

            loop_cm = tc.For_i(0, repeat, 1) if repeat > 1 else nullcontext()
            with loop_cm:
                # ---- Phase A: feat path, four 1024-column quarters.  Moving
                # operand fp8 (x16), stationary bf16 (/16) so products are
                # exact.  2 PSUM banks per quarter, pool bufs=2: quarter k+1's
                # matmuls run while quarter k's PSUM drains to SBUF.
                with tc.tile_pool(name="psum_a", bufs=2, space="PSUM") as psum_a:
                    for qq in range(4):
                        ps1 = psum_a.tile([128, 2, 512], F32, tag="ps1")
                        for dc in range(4):
                            ft = ftp.tile([128, 1024], F8, tag="ft")
                            dma_eng = nc.sync if dc % 2 == 0 else nc.scalar
                            if do_dma:
                                dma_eng.dma_start(
                                    out=ft,
                                    in_=featT[dc * 128 : (dc + 1) * 128,
                                              qq * 1024 : (qq + 1) * 1024],
                                )
                            elif do_mm:
                                dma_eng.dma_start(
                                    out=ft[:, 0:16],
                                    in_=featT[dc * 128 : (dc + 1) * 128, 0:16],
                                )
                            for qw in range(2):
                                if not do_mm:
                                    break
                                nc.tensor.matmul(
                                    ps1[:, qw, :], neT_sb[:, dc, :],
                                    ft[:, qw * 512 : (qw + 1) * 512],
                                    start=(dc == 0), stop=(dc == 3),
                                )
                        if do_stats:
                            # park scores1 in SBUF so the banks free fast
                            nc.vector.tensor_copy(
                                out=s1c_sb[:, qq * 2 : (qq + 1) * 2, :], in_=ps1
                            )

                # ---- Phase B: logit path (fp8 DoubleRow), two column halves
                # with separate 4-bank accumulators; half 0's stats run under
                # half 1's DMA/matmul stream.
                with (
                    tc.tile_pool(name="tcp", bufs=24) as tcp,
                    tc.tile_pool(name="psum_b", bufs=1, space="PSUM") as psum_b,
                ):
                    psB = [psum_b.tile([128, 4, 512], F32, tag=f"psB{h}",
                                       name=f"psB{h}")
                           for h in range(2)]

                    def b_step(h, cb):
                        tcb = tcp.tile([128, 2, QH], F8, tag="tcb", name="tcb")
                        dma_eng = nc.sync if cb % 2 == 0 else nc.scalar
                        if do_dma:
                            dma_eng.dma_start(out=tcb, in_=logitT[h, cb, :, :, :])
                        elif do_mm:
                            dma_eng.dma_start(
                                out=tcb[:, :, 0:16], in_=logitT[h, cb, :, :, 0:16]
                            )
                        for qw in range(4):
                            if not do_mm:
                                break
                            nc.tensor.matmul(
                                psB[h][:, qw, :],
                                nlT_sb[:, 2 * cb : 2 * cb + 2, :],
                                tcb[:, :, qw * 512 : (qw + 1) * 512],
                                start=(cb == 0), stop=(cb == CB2 - 1),
                                perf_mode=DR,
                            )

                    for cb in range(12):
                        b_step(0, cb)
                    if do_stats:
                        # deferred phase A stats, interleaved mid-stream so the
                        # DVE/ACT work hides under the logit DMA stream
                        stats_block(s1c_sb, 0, 8, A_CHUNKS)
                    for cb in range(12, CB2):
                        b_step(0, cb)
                    for cb in range(6):
                        b_step(1, cb)
                    if do_stats:
                        # half-0 stats while half 1 streams
                        stats_block(psB[0], 32, 64, 4)
                    for cb in range(6, CB2):
                        b_step(1, cb)
                    if do_stats:
                        stats_block(psB[1], 36, 68, 4)

            nc.sync.dma_start(out=stats[:, :], in_=out_sb)

    _split_excess_waits(nc)
    return nc


_PROGRAM: bass.Bass | None = None
LAST_RESULTS = None  # BassKernelResults of the most recent run (for profiling)


def _get_program() -> bass.Bass:
    global _PROGRAM
    if _PROGRAM is None:
        _PROGRAM = _build_program()
    return _PROGRAM


def _to_f8(t):
    """torch f32 tensor -> numpy ml_dtypes.float8_e4m3 view, same shape.
    torch's e4m3fn and TRN/ml_dtypes e4m3 agree bit-for-bit for |x| <= 240;
    all tensors quantized here are well inside that."""
    import ml_dtypes
    import torch

    return (
        t.to(torch.float8_e4m3fn).view(torch.int8).numpy()
        .view(ml_dtypes.float8_e4m3)
    )


def host_prep(old_embeds, old_logits, new_embeds, new_logits, labels,
              feat_queue, logit_queue, queue_labels, header):
    """Scatter + normalize + quantize + pre-transpose on host; evaluates the
    sparse masked sums exactly in float64.  Returns per-core in_maps and
    (W, A1, A2, M) host vectors."""
    import torch

    old_embeds = np.asarray(old_embeds, dtype=np.float32)
    old_logits = np.asarray(old_logits, dtype=np.float32)
    new_embeds = np.asarray(new_embeds, dtype=np.float32)
    new_logits = np.asarray(new_logits, dtype=np.float32)
    feat_queue = np.array(feat_queue, dtype=np.float32)   # copies (scattered below)
    logit_queue = np.array(logit_queue, dtype=np.float32)
    labels_np = np.asarray(labels).astype(np.int64)
    queue_labels_np = np.asarray(queue_labels).astype(np.int64)
    hdr = int(np.asarray(header))

    n = old_embeds.shape[0]
    q = feat_queue.shape[0]
    assert (n, q) == (N, Q)

    # circular queue scatter
    idx = (hdr + np.arange(n)) % q
    feat_queue[idx] = old_embeds
    logit_queue[idx] = old_logits
    queue_labels_np[idx] = labels_np

    # normalize new_embeds (f64 intermediate, f32 result)
    ne64 = new_embeds.astype(np.float64)
    norm = np.sqrt((ne64 * ne64).sum(axis=1, keepdims=True))
    new_e = (ne64 / np.maximum(norm, EPS)).astype(np.float32)

    # ---- sparse part (exact, host): the label mask has ~Q/C positives per
    # row, so  W = sum_j mask*w,  A = sum_j mask*w*s  are ~512 short dot
    # products in float64.
    mask = queue_labels_np[None, :] == labels_np[:, None]
    M = mask.sum(axis=1).astype(np.float64)               # [N], >= 1 by construction
    rows, cols = np.nonzero(mask)
    w_v = 0.5 * ((old_embeds[rows].astype(np.float64)
                  * feat_queue[cols].astype(np.float64)).sum(axis=1) + 1.0)
    s1_v = (new_e[rows].astype(np.float64)
            * feat_queue[cols].astype(np.float64)).sum(axis=1)
    s2_v = (new_logits[rows].astype(np.float64)
            * logit_queue[cols].astype(np.float64)).sum(axis=1)
    W = np.bincount(rows, weights=w_v, minlength=N)
    A1 = np.bincount(rows, weights=w_v * s1_v, minlength=N)
    A2 = np.bincount(rows, weights=w_v * s2_v, minlength=N)

    def _sbuf_image(aT):
        # [K, N] -> [128, K//128, N] partition-major SBUF image
        k = aT.shape[0]
        return np.ascontiguousarray(
            aT.reshape(k // 128, 128, aT.shape[1]).transpose(1, 0, 2)
        )

    import ml_dtypes

    neT = _sbuf_image(np.ascontiguousarray((new_e / FSCALE).T)).astype(
        ml_dtypes.bfloat16)                                   # [128, 4, N] bf16
    nlT_t = torch.from_numpy(new_logits).t().contiguous()     # [C, N]
    nlT = _sbuf_image(_to_f8(nlT_t))                          # [128, 64, N] fp8

    # fp8 queue shards.  featT is scaled x16 so elements (~N(0, 1/512)) land
    # in e4m3's normal range; the bf16 stationary above carries the /16.
    fq_t = torch.from_numpy(feat_queue)
    lq8 = torch.from_numpy(logit_queue).to(torch.float8_e4m3fn)  # [Q, C]

    in_maps = []
    for d in range(N_CORES):
        sl = slice(d * QS, (d + 1) * QS)
        featT = _to_f8((fq_t[sl].t() * FSCALE).contiguous())     # [D, QS] fp8
        # logit shard -> DoubleRow moving layout [2, CB2, 128, 2, QH]:
        # [h, c2, p, r, j] = logit_queue[qs0 + h*QH + j, c2*256 + r*128 + p]
        lsh = lq8[sl].view(torch.int8).t().contiguous()          # [C, QS] i8
        ldr = (lsh.reshape(CB2, 2, 128, 2, QH).permute(3, 0, 2, 1, 4)
               .contiguous().numpy().view(ml_dtypes.float8_e4m3))
        in_maps.append({
            "featT": featT,
            "logitT": ldr,
            "neT": neT,
            "nlT": nlT,
        })
    return in_maps, (W, A1, A2, M)


def combine_stats(parts: np.ndarray, host_sums):
    """parts: [n_cores, 128, 128] f32 stats tiles + exact host sparse sums
    -> (l1, l2) f32 scalars."""
    W, A1, A2, M = host_sums
    parts = parts.astype(np.float64)
    m1p = parts[:, :, 0:8]
    z1p = parts[:, :, 8:16]
    m2p = parts[:, :, 32:40]
    z2p = parts[:, :, 64:72]

    m1 = m1p.max(axis=(0, 2))
    m2 = m2p.max(axis=(0, 2))
    Z1 = (z1p * np.exp(m1p - m1[None, :, None])).sum(axis=(0, 2))
    Z2 = (z2p * np.exp(m2p - m2[None, :, None])).sum(axis=(0, 2))

    # sum_j maskw * log_prob = A - (m + log Z) * W ; divide by count, mean, negate
    l1 = -np.mean((A1 - (m1 + np.log(Z1)) * W) / M)
    l2 = -np.mean((A2 - (m2 + np.log(Z2)) * W) / M)
    return (np.float32(l1), np.float32(l2))


def kernel(old_embeds, old_logits, new_embeds, new_logits, labels,
           feat_queue, logit_queue, queue_labels, header):
    global LAST_RESULTS
    in_maps, host_sums = host_prep(
        old_embeds, old_logits, new_embeds, new_logits, labels,
        feat_queue, logit_queue, queue_labels, header,
    )
    nc = _get_program()
    LAST_RESULTS = run_bass_kernel_spmd(nc, in_maps, list(range(N_CORES)))
    parts = np.stack([LAST_RESULTS.results[d]["stats"] for d in range(N_CORES)])
    return combine_stats(parts, host_sums)
